# revision 34
# baseline (speedup 1.0000x reference)
"""Trainium2 Bass kernel for nn_BaseQuantizer (multiscale residual VQ).

Data-parallel over batch B=64 across 8 NeuronCores (8 images per core).
Per core, for each of 5 levels (pn in 1,2,4,8,16):
  - area-downsample residual (exact block mean) via DVE strided reduces
  - distance argmin over 4096 codes: PE matmul computes
      s[r,k] = zd[r]·w[k] - |w[k]|^2/2   (argmax s == argmin L2 dist)
    via an augmented K=33 contraction ([zd;1] x [w^T; -wsq/2]); DVE
    max/max_index extract the first-index argmax per row.
  - code gather via indirect DMA from the replicated codebook in DRAM
  - bicubic upsample via PE matmuls against precomputed (A (x) A) maps
  - residual update + accumulation, ret[l] written out per level.
"""

import os
import sys

import numpy as np

if "/opt/trn_rl_repo" not in sys.path:
    sys.path.insert(0, "/opt/trn_rl_repo")

import concourse.bass as bass
import concourse.bacc as bacc
import concourse.mybir as mybir
import concourse.tile as tile
from concourse.bass_utils import run_bass_kernel_spmd

F32 = mybir.dt.float32
F32R = mybir.dt.float32r  # full-rate (1 cyc/col) exact-fp32 matmul path
U32 = mybir.dt.uint32
AX = mybir.AxisListType
ALU = mybir.AluOpType
ACTF = mybir.ActivationFunctionType

TRACE = False  # set True (e.g. from test.py) to capture an NTFF profile
LAST_RESULTS = None

N_CORES = 8
B_FULL = 64
B = B_FULL // N_CORES  # 8 images per core
C = 32
H = W = 16
S = H * W  # 256
K_CODES = 4096
MS = (1, 2, 4, 8, 16)
LEVELS = len(MS)


# ---------------------------------------------------------------------------
# Host-side constants: bicubic (jax.image.resize method='cubic') weight maps.
# ---------------------------------------------------------------------------
def _keys_cubic(x: np.ndarray, a: float = -0.5) -> np.ndarray:
    x = np.abs(x)
    out = np.where(
        x <= 1.0,
        ((a + 2.0) * x - (a + 3.0)) * x * x + 1.0,
        np.where(x < 2.0, (((x - 5.0) * x + 8.0) * x - 4.0) * a, 0.0),
    )
    return out


def _resize_weight_mat(in_size: int, out_size: int) -> np.ndarray:
    """Replicates jax.image.compute_weight_mat for method='cubic' upsampling.

    Returns [in_size, out_size]; out = wm.T @ in along the resized axis.
    """
    scale = out_size / in_size
    sample_f = (np.arange(out_size, dtype=np.float64) + 0.5) / scale - 0.5
    x = np.abs(sample_f[None, :] - np.arange(in_size, dtype=np.float64)[:, None])
    weights = _keys_cubic(x)
    total = weights.sum(axis=0)
    weights = np.where(total[None, :] > 0, weights / total[None, :], 0.0)
    in_bounds = (sample_f >= -0.5) & (sample_f <= in_size - 0.5)
    weights = weights * in_bounds[None, :]
    return weights.astype(np.float32)


def _wup_matrix(pn: int) -> np.ndarray:
    """[pn*pn, 256] map: z_up[(y,x)] = sum_{(i,j)} q[(i,j)] * Wup[(i,j),(y,x)]."""
    wm = _resize_weight_mat(pn, H)  # [pn, 16]
    wup = np.einsum("iy,jx->ijyx", wm, wm).reshape(pn * pn, H * W)
    return np.ascontiguousarray(wup.astype(np.float32))


# ---------------------------------------------------------------------------
# Device kernel
# ---------------------------------------------------------------------------
def build_vq_kernel(tc, out_ap, z_ap, emb_ap, emb33_ap, wup_aps, ident_ap):
    """out_ap: [5, B, 32, 256]; z_ap: [B, 32, 256]; emb_ap: [4096, 32];
    emb33_ap: [4096, 33] (emb ++ -wsq/2 col, for candidate re-rank);
    wup_aps: dict pn -> [pn*pn, 256]; ident_ap: [128, 128]."""
    nc = tc.nc
    BF16 = mybir.dt.bfloat16
    BS = B * S  # 2048
    NCAND = 10  # 5 coarse candidates per 2048-wide half

    from contextlib import ExitStack

    with ExitStack() as ctx:
        const = ctx.enter_context(tc.tile_pool(name="const", bufs=1))
        psum = ctx.enter_context(tc.tile_pool(name="psum", bufs=2, space="PSUM"))
        work = ctx.enter_context(tc.tile_pool(name="work", bufs=2))

        # Persistent state tiles
        w2b = const.tile([C + 1, K_CODES], BF16, tag="w2b")    # [w^T ; -wsq/2] bf16
        zrest = const.tile([C + 1, BS], F32, tag="zrest")      # row 32 == 1.0
        zdec = const.tile([C, BS], F32, tag="zdec")
        ident = const.tile([128, 128], F32, tag="ident")
        wup = {
            pn: const.tile([pn * pn, S], F32, tag=f"wup{pn}", name=f"wup{pn}")
            for pn in MS[1:-1]
        }
        neghalf = const.tile([C, 1], F32, tag="neghalf")

        # Loads and initialization
        nc.sync.dma_start(ident[:], ident_ap[:])
        for pn in MS[1:-1]:
            nc.sync.dma_start(wup[pn][:], wup_aps[pn][:])
        nc.sync.dma_start(
            zrest[0:C, :].rearrange("c (b s) -> c b s", b=B),
            z_ap.rearrange("b c s -> c b s"),
        )
        nc.vector.memset(zrest[C : C + 1, :], 1.0)
        nc.vector.memset(zdec[:], 0.0)
        nc.vector.memset(neghalf[:], -0.5)

        # --- codebook setup: wT (f32 scratch) via PE transposes, then bf16 --
        wnat = work.tile([128, 32 * C], F32, tag="wnat")  # [128, (chunk, c)]
        nc.sync.dma_start(
            wnat[:].rearrange("p (ch c) -> p ch c", c=C),
            emb_ap.rearrange("(ch p) c -> p ch c", p=128),
        )
        wtf = work.tile([C, K_CODES], F32, tag="wtf")
        for h in range(2):
            ps = psum.tile([128, 2048], F32, tag="ps")
            for i in range(16):
                chk = h * 16 + i
                nc.tensor.transpose(
                    ps[0:C, i * 128 : (i + 1) * 128],
                    wnat[:, chk * C : (chk + 1) * C],
                    ident[:],
                )
            nc.scalar.copy(wtf[0:C, h * 2048 : (h + 1) * 2048], ps[0:C, :])
        nc.vector.tensor_copy(w2b[0:C, :], wtf[:, :])

        # w2b[32] = -wsq/2 via ACT square + PE ones-matmul (out [1, N])
        sq = work.tile([C, K_CODES], F32, tag="sq")
        nc.scalar.activation(sq[:], wtf[:, :], ACTF.Square)
        for h in range(2):
            ps = psum.tile([128, 2048], F32, tag="ps")
            for kk in range(4):
                nc.tensor.matmul(
                    ps[0:1, kk * 512 : (kk + 1) * 512],
                    lhsT=neghalf[:],
                    rhs=sq[:, h * 2048 + kk * 512 : h * 2048 + (kk + 1) * 512],
                    start=True,
                    stop=True,
                )
            nc.scalar.copy(w2b[C : C + 1, h * 2048 : (h + 1) * 2048], ps[0:1, :])

        # --- level loop ----------------------------------------------------
        for lvl, pn in enumerate(MS):
            pp = pn * pn
            rows_l = B * pp
            blk = H // pn
            last = lvl == LEVELS - 1

            if not last:
                # exact area pooling: two strided sum-reduces + scale
                zd = work.tile([C + 1, 512], F32, tag="zd")
                t1 = work.tile([C, 128 * pn], F32, tag="t1")
                nc.vector.reduce_sum(
                    t1[:, : 128 * pn],
                    zrest[0:C, :].rearrange(
                        "c (bh pw bw) -> c bh pw bw", pw=pn, bw=blk
                    ),
                    axis=AX.X,
                )
                nc.vector.reduce_sum(
                    zd[0:C, 0:rows_l],
                    t1[:, : 128 * pn].rearrange(
                        "c (b ph bh2 pw) -> c b ph pw bh2", ph=pn, bh2=blk, pw=pn
                    ),
                    axis=AX.X,
                )
                nc.vector.tensor_scalar_mul(
                    zd[0:C, 0:rows_l], zd[0:C, 0:rows_l], 1.0 / float(blk * blk)
                )
                nc.vector.memset(zd[C : C + 1, 0:rows_l], 1.0)
                lhs_f32 = zd
            else:
                lhs_f32 = zrest

            n_chunks = (rows_l + 127) // 128

            # bf16 copy of the augmented lhsT for the coarse matmul
            lhs_b = work.tile([C + 1, BS], BF16, tag="lhsb")
            nc.scalar.copy(lhs_b[:, 0:rows_l], lhs_f32[:, 0:rows_l])

            # zdT: row-major f32 copies of each chunk's [rows, 33] transpose
            # (for the exact re-rank dot products), chunk chv at col 64*chv.
            zdT = work.tile([128, 64 * 16], F32, tag="zdT")
            tps = psum.tile([128, 2048], F32, tag="ps")
            for chv in range(n_chunks):
                r0 = chv * 128
                rows = min(128, rows_l - r0)
                nc.tensor.transpose(
                    tps[0:rows, 64 * chv : 64 * chv + C + 1],
                    lhs_f32[:, r0 : r0 + rows],
                    ident[0 : C + 1, 0 : C + 1],
                )
                nc.scalar.copy(
                    zdT[0:rows, 64 * chv : 64 * chv + C + 1],
                    tps[0:rows, 64 * chv : 64 * chv + C + 1],
                )
            if 0 < lvl < LEVELS - 1:
                tokT = work.tile([64, B], U32, tag="tokT")
                q = work.tile([64, B * C], F32, tag="q")
            if last:
                g4 = work.tile([128, 16 * C], F32, tag="g4")

            for chv in range(n_chunks):
                r0 = chv * 128
                rows = min(128, rows_l - r0)

                tok = work.tile([128, 1], U32, tag="tok")
                idxs = []
                for hf in range(2):
                    ps = psum.tile([128, 2048], F32, tag="ps")
                    for kk in range(4):
                        nc.tensor.matmul(
                            ps[0:rows, kk * 512 : (kk + 1) * 512],
                            lhsT=lhs_b[:, r0 : r0 + rows],
                            rhs=w2b[
                                :, hf * 2048 + kk * 512 : hf * 2048 + (kk + 1) * 512
                            ],
                            start=True,
                            stop=True,
                        )
                    top8 = work.tile([128, 8], F32, tag="top8")
                    nc.vector.max(top8[0:rows, :], ps[0:rows, :])
                    idx8 = work.tile([128, 8], U32, tag="idx8")
                    nc.vector.max_index(idx8[0:rows, :], top8[0:rows, :], ps[0:rows, :])
                    idxs.append(idx8)

                # exact fp32 re-rank of the top-5 coarse candidates per half
                gidxu = work.tile([128, NCAND], U32, tag="gidxu")
                nc.vector.tensor_copy(gidxu[0:rows, 0:5], idxs[0][0:rows, 0:5])
                nc.vector.tensor_scalar(
                    gidxu[0:rows, 5:10], idxs[1][0:rows, 0:5], 2048, None, op0=ALU.add
                )
                gidxf = work.tile([128, NCAND], F32, tag="gidxf")
                nc.vector.tensor_copy(gidxf[0:rows, :], gidxu[0:rows, :])
                cand = work.tile([128, NCAND * (C + 1)], F32, tag="cand")
                for j in range(NCAND):
                    nc.gpsimd.indirect_dma_start(
                        out=cand[0:rows, j * (C + 1) : (j + 1) * (C + 1)],
                        out_offset=None,
                        in_=emb33_ap[:],
                        in_offset=bass.IndirectOffsetOnAxis(
                            ap=gidxu[0:rows, j : j + 1], axis=0
                        ),
                    )
                prod = work.tile([128, NCAND * (C + 1)], F32, tag="prod")
                zbc = (
                    zdT[0:rows, 64 * chv : 64 * chv + C + 1]
                    .rearrange("p (o c) -> p o c", o=1)
                    .broadcast_to([rows, NCAND, C + 1])
                )
                nc.vector.tensor_tensor(
                    prod[0:rows, :].rearrange("p (t c) -> p t c", c=C + 1),
                    cand[0:rows, :].rearrange("p (t c) -> p t c", c=C + 1),
                    zbc,
                    op=ALU.mult,
                )
                s10 = work.tile([128, NCAND], F32, tag="s10")
                nc.vector.tensor_reduce(
                    s10[0:rows, :],
                    prod[0:rows, :].rearrange("p (t c) -> p t c", c=C + 1),
                    axis=AX.X,
                    op=ALU.add,
                )
                m1 = work.tile([128, 1], F32, tag="m1")
                nc.vector.tensor_reduce(
                    m1[0:rows, :], s10[0:rows, :], axis=AX.X, op=ALU.max
                )
                onehn = work.tile([128, NCAND], F32, tag="onehn")
                nc.vector.tensor_scalar(
                    onehn[0:rows, :], s10[0:rows, :], m1[0:rows, 0:1], None,
                    op0=ALU.is_lt,
                )
                tsel = work.tile([128, NCAND], F32, tag="tsel")
                nc.vector.scalar_tensor_tensor(
                    tsel[0:rows, :], onehn[0:rows, :], 8192.0, gidxf[0:rows, :],
                    op0=ALU.mult, op1=ALU.add,
                )
                tokf = work.tile([128, 1], F32, tag="tokf")
                nc.vector.tensor_reduce(
                    tokf[0:rows, :], tsel[0:rows, :], axis=AX.X, op=ALU.min
                )
                nc.vector.tensor_copy(tok[0:rows, :], tokf[0:rows, :])

                if lvl == 0:
                    # 1x1 level: gather the 8 codes, transpose to [C, B],
                    # then broadcast-subtract/add over the 256 pixels.
                    g0 = work.tile([B, C], F32, tag="g")
                    nc.gpsimd.indirect_dma_start(
                        out=g0[:, :],
                        out_offset=None,
                        in_=emb_ap[:],
                        in_offset=bass.IndirectOffsetOnAxis(ap=tok[0:B, 0:1], axis=0),
                    )
                    tp = psum.tile([128, 2048], F32, tag="ps")
                    nc.tensor.transpose(tp[0:C, 0:B], g0[:, :], ident[0:B, 0:B])
                    q0t = work.tile([C, B], F32, tag="q0t")
                    nc.vector.tensor_copy(q0t[:], tp[0:C, 0:B])
                    bc = q0t[:].rearrange("c (b one) -> c b one", one=1).broadcast_to(
                        [C, B, S]
                    )
                    zr3 = zrest[0:C, :].rearrange("c (b s) -> c b s", b=B)
                    zd3 = zdec[:, :].rearrange("c (b s) -> c b s", b=B)
                    nc.vector.tensor_sub(zr3, zr3, bc)
                    nc.vector.tensor_add(zd3, zd3, bc)
                elif not last:
                    # reorder tokens [(b,pp),1] -> tokT [pp, b]
                    b_per = rows // pp
                    for bb in range(b_per):
                        b_g = chv * b_per + bb
                        nc.sync.dma_start(
                            tokT[0:pp, b_g : b_g + 1],
                            tok[bb * pp : (bb + 1) * pp, 0:1],
                        )
                else:
                    # gather this chunk's codes into the level-wide staging
                    # tile; transposes happen once after all chunks.
                    nc.gpsimd.indirect_dma_start(
                        out=g4[:, chv * C : (chv + 1) * C],
                        out_offset=None,
                        in_=emb_ap[:],
                        in_offset=bass.IndirectOffsetOnAxis(ap=tok[:, 0:1], axis=0),
                    )

            if 0 < lvl < LEVELS - 1:
                for b_g in range(B):
                    nc.gpsimd.indirect_dma_start(
                        out=q[0:pp, b_g * C : (b_g + 1) * C],
                        out_offset=None,
                        in_=emb_ap[:],
                        in_offset=bass.IndirectOffsetOnAxis(
                            ap=tokT[0:pp, b_g : b_g + 1], axis=0
                        ),
                    )
                up = psum.tile([128, 2048], F32, tag="ps")
                for b_g in range(B):
                    nc.tensor.matmul(
                        up[0:C, b_g * S : (b_g + 1) * S],
                        lhsT=q[0:pp, b_g * C : (b_g + 1) * C],
                        rhs=wup[pn][:],
                        start=True,
                        stop=True,
                    )
                nc.vector.tensor_sub(zrest[0:C, :], zrest[0:C, :], up[0:C, :])
                nc.vector.tensor_add(zdec[:, :], zdec[:, :], up[0:C, :])
            elif last:
                tp = psum.tile([128, 2048], F32, tag="ps")
                for chv in range(16):
                    nc.tensor.transpose(
                        tp[0:C, chv * 128 : (chv + 1) * 128],
                        g4[:, chv * C : (chv + 1) * C],
                        ident[:],
                    )
                nc.vector.tensor_add(zdec[:, :], zdec[:, :], tp[0:C, :])

            nc.sync.dma_start(
                out_ap[lvl : lvl + 1].rearrange("o b c s -> c (o b) s"),
                zdec[:].rearrange("c (b s) -> c b s", b=B),
            )


# ---------------------------------------------------------------------------
# Host entry point
# ---------------------------------------------------------------------------
def _build_nc():
    nc = bacc.Bacc("TRN2", target_bir_lowering=False, debug=False)
    z_t = nc.dram_tensor("z", [B, C, S], F32, kind="ExternalInput")
    emb_t = nc.dram_tensor("emb", [K_CODES, C], F32, kind="ExternalInput")
    emb33_t = nc.dram_tensor("emb33", [K_CODES, C + 1], F32, kind="ExternalInput")
    ident_t = nc.dram_tensor("ident", [128, 128], F32, kind="ExternalInput")
    wup_ts = {
        pn: nc.dram_tensor(f"wup{pn}", [pn * pn, S], F32, kind="ExternalInput")
        for pn in MS[1:-1]
    }
    out_t = nc.dram_tensor("out", [LEVELS, B, C, S], F32, kind="ExternalOutput")

    with tile.TileContext(nc) as tc:
        build_vq_kernel(
            tc,
            out_t.ap(),
            z_t.ap(),
            emb_t.ap(),
            emb33_t.ap(),
            {pn: t.ap() for pn, t in wup_ts.items()},
            ident_t.ap(),
        )
    nc.compile()
    return nc


def kernel(z_enc: np.ndarray, emb_weight: np.ndarray) -> np.ndarray:
    z_enc = np.ascontiguousarray(z_enc, dtype=np.float32)
    emb_weight = np.ascontiguousarray(emb_weight, dtype=np.float32)
    assert z_enc.shape == (B_FULL, C, H, W)
    assert emb_weight.shape == (K_CODES, C)

    nc = _build_nc()

    ident = np.eye(128, dtype=np.float32)
    wups = {pn: _wup_matrix(pn) for pn in MS[1:-1]}
    wsq = (emb_weight.astype(np.float64) ** 2).sum(axis=1)
    emb33 = np.concatenate(
        [emb_weight, (-0.5 * wsq[:, None]).astype(np.float32)], axis=1
    ).astype(np.float32)

    z_flat = z_enc.reshape(B_FULL, C, S)
    in_maps = []
    for core in range(N_CORES):
        shard = np.ascontiguousarray(z_flat[core * B : (core + 1) * B])
        m = {"z": shard, "emb": emb_weight, "emb33": emb33, "ident": ident}
        for pn in MS[1:-1]:
            m[f"wup{pn}"] = wups[pn]
        in_maps.append(m)

    res = run_bass_kernel_spmd(nc, in_maps, list(range(N_CORES)), trace=TRACE)
    if TRACE:
        global LAST_RESULTS
        LAST_RESULTS = res
    outs = [res.results[core]["out"] for core in range(N_CORES)]
    full = np.concatenate(outs, axis=1)  # [5, 64, 32, 256]
    return full.reshape(LEVELS, B_FULL, C, H, W)


if __name__ == "__main__":
    z = np.random.randn(B_FULL, C, H, W).astype(np.float32)
    emb = (np.random.randn(K_CODES, C) * 0.01).astype(np.float32)
    out = kernel(z, emb)
    print("kernel output:", out.shape, out.dtype)


# revision 39
# speedup vs baseline: 1.4794x; 1.4794x over previous
"""Trainium2 Bass kernel for nn_BaseQuantizer (multiscale residual VQ).

Data-parallel over batch B=64 across 8 NeuronCores (8 images per core).
Per core, for each of 5 levels (pn in 1,2,4,8,16):
  - area-downsample residual (exact block mean) via DVE strided reduces
  - distance argmin over 4096 codes: PE matmul computes
      s[r,k] = zd[r]·w[k] - |w[k]|^2/2   (argmax s == argmin L2 dist)
    via an augmented K=33 contraction ([zd;1] x [w^T; -wsq/2]); DVE
    max/max_index extract the first-index argmax per row.
  - code gather via indirect DMA from the replicated codebook in DRAM
  - bicubic upsample via PE matmuls against precomputed (A (x) A) maps
  - residual update + accumulation, ret[l] written out per level.
"""

import os
import sys

import numpy as np

if "/opt/trn_rl_repo" not in sys.path:
    sys.path.insert(0, "/opt/trn_rl_repo")

import concourse.bass as bass
import concourse.bacc as bacc
import concourse.mybir as mybir
import concourse.tile as tile
from concourse.bass_utils import run_bass_kernel_spmd

F32 = mybir.dt.float32
F32R = mybir.dt.float32r  # full-rate (1 cyc/col) exact-fp32 matmul path
U32 = mybir.dt.uint32
AX = mybir.AxisListType
ALU = mybir.AluOpType
ACTF = mybir.ActivationFunctionType

TRACE = False  # set True (e.g. from test.py) to capture an NTFF profile
LAST_RESULTS = None

N_CORES = 8
B_FULL = 64
B = B_FULL // N_CORES  # 8 images per core
C = 32
H = W = 16
S = H * W  # 256
K_CODES = 4096
MS = (1, 2, 4, 8, 16)
LEVELS = len(MS)


# ---------------------------------------------------------------------------
# Host-side constants: bicubic (jax.image.resize method='cubic') weight maps.
# ---------------------------------------------------------------------------
def _keys_cubic(x: np.ndarray, a: float = -0.5) -> np.ndarray:
    x = np.abs(x)
    out = np.where(
        x <= 1.0,
        ((a + 2.0) * x - (a + 3.0)) * x * x + 1.0,
        np.where(x < 2.0, (((x - 5.0) * x + 8.0) * x - 4.0) * a, 0.0),
    )
    return out


def _resize_weight_mat(in_size: int, out_size: int) -> np.ndarray:
    """Replicates jax.image.compute_weight_mat for method='cubic' upsampling.

    Returns [in_size, out_size]; out = wm.T @ in along the resized axis.
    """
    scale = out_size / in_size
    sample_f = (np.arange(out_size, dtype=np.float64) + 0.5) / scale - 0.5
    x = np.abs(sample_f[None, :] - np.arange(in_size, dtype=np.float64)[:, None])
    weights = _keys_cubic(x)
    total = weights.sum(axis=0)
    weights = np.where(total[None, :] > 0, weights / total[None, :], 0.0)
    in_bounds = (sample_f >= -0.5) & (sample_f <= in_size - 0.5)
    weights = weights * in_bounds[None, :]
    return weights.astype(np.float32)


def _wup_matrix(pn: int) -> np.ndarray:
    """[pn*pn, 256] map: z_up[(y,x)] = sum_{(i,j)} q[(i,j)] * Wup[(i,j),(y,x)]."""
    wm = _resize_weight_mat(pn, H)  # [pn, 16]
    wup = np.einsum("iy,jx->ijyx", wm, wm).reshape(pn * pn, H * W)
    return np.ascontiguousarray(wup.astype(np.float32))


# ---------------------------------------------------------------------------
# Device kernel
# ---------------------------------------------------------------------------
def build_vq_kernel(tc, out_ap, z_ap, emb_ap, wup_aps, ident_ap):
    """out_ap: [5, B, 32, 256]; z_ap: [B, 32, 256]; emb_ap: [4096, 32];
    wup_aps: dict pn -> [pn*pn, 256]; ident_ap: [128, 128]."""
    nc = tc.nc
    BF16 = mybir.dt.bfloat16
    BS = B * S  # 2048

    from contextlib import ExitStack

    with ExitStack() as ctx:
        const = ctx.enter_context(tc.tile_pool(name="const", bufs=1))
        psum = ctx.enter_context(tc.tile_pool(name="psum", bufs=2, space="PSUM"))
        work = ctx.enter_context(tc.tile_pool(name="work", bufs=2))

        # Persistent state tiles
        w2h = const.tile([C + 1, K_CODES], BF16, tag="w2h")    # bf16 hi of [w^T; -wsq/2]
        w2l = const.tile([C + 1, K_CODES], BF16, tag="w2l")    # bf16 lo residual
        zrest = const.tile([C + 1, BS], F32, tag="zrest")      # row 32 == 1.0
        zdec = const.tile([C, BS], F32, tag="zdec")
        ident = const.tile([128, 128], F32, tag="ident")
        wup = {
            pn: const.tile([pn * pn, S], F32, tag=f"wup{pn}", name=f"wup{pn}")
            for pn in MS[1:-1]
        }
        neghalf = const.tile([C, 1], F32, tag="neghalf")

        # Loads and initialization
        nc.sync.dma_start(ident[:], ident_ap[:])
        for pn in MS[1:-1]:
            nc.sync.dma_start(wup[pn][:], wup_aps[pn][:])
        nc.sync.dma_start(
            zrest[0:C, :].rearrange("c (b s) -> c b s", b=B),
            z_ap.rearrange("b c s -> c b s"),
        )
        nc.vector.memset(zrest[C : C + 1, :], 1.0)
        nc.vector.memset(zdec[:], 0.0)
        nc.vector.memset(neghalf[:], -0.5)

        # --- codebook setup: wT (f32 scratch) via PE transposes, then bf16 --
        wnat = work.tile([128, 32 * C], F32, tag="wnat", bufs=1)  # [128, (chunk, c)]
        nc.sync.dma_start(
            wnat[:].rearrange("p (ch c) -> p ch c", c=C),
            emb_ap.rearrange("(ch p) c -> p ch c", p=128),
        )
        wtf = work.tile([C + 1, K_CODES], F32, tag="wtf", bufs=1)
        for h in range(2):
            ps = psum.tile([128, 2048], F32, tag="ps")
            for i in range(16):
                chk = h * 16 + i
                nc.tensor.transpose(
                    ps[0:C, i * 128 : (i + 1) * 128],
                    wnat[:, chk * C : (chk + 1) * C],
                    ident[:],
                )
            nc.scalar.copy(wtf[0:C, h * 2048 : (h + 1) * 2048], ps[0:C, :])

        # wsq row (f32) via ACT square + PE ones-matmul (out [1, N])
        sq = work.tile([C, K_CODES], F32, tag="sq", bufs=1)
        nc.scalar.activation(sq[:], wtf[0:C, :], ACTF.Square)
        for h in range(2):
            ps = psum.tile([128, 2048], F32, tag="ps")
            for kk in range(4):
                nc.tensor.matmul(
                    ps[0:1, kk * 512 : (kk + 1) * 512],
                    lhsT=neghalf[:],
                    rhs=sq[:, h * 2048 + kk * 512 : h * 2048 + (kk + 1) * 512],
                    start=True,
                    stop=True,
                )
            nc.scalar.copy(wtf[C : C + 1, h * 2048 : (h + 1) * 2048], ps[0:1, :])

        # hi/lo bf16 split of the augmented codebook
        nc.scalar.copy(w2h[:, :], wtf[:, :])
        wres = work.tile([C + 1, K_CODES], F32, tag="wres", bufs=1)
        nc.vector.tensor_sub(wres[:, :], wtf[:, :], w2h[:, :])
        nc.scalar.copy(w2l[:, :], wres[:, :])

        # --- level loop ----------------------------------------------------
        for lvl, pn in enumerate(MS):
            pp = pn * pn
            rows_l = B * pp
            blk = H // pn
            last = lvl == LEVELS - 1

            if not last:
                # exact area pooling: two strided sum-reduces + scale
                zd = work.tile([C + 1, 512], F32, tag="zd", bufs=1)
                t1 = work.tile([C, 128 * pn], F32, tag="t1", bufs=1)
                nc.vector.reduce_sum(
                    t1[:, : 128 * pn],
                    zrest[0:C, :].rearrange(
                        "c (bh pw bw) -> c bh pw bw", pw=pn, bw=blk
                    ),
                    axis=AX.X,
                )
                nc.vector.reduce_sum(
                    zd[0:C, 0:rows_l],
                    t1[:, : 128 * pn].rearrange(
                        "c (b ph bh2 pw) -> c b ph pw bh2", ph=pn, bh2=blk, pw=pn
                    ),
                    axis=AX.X,
                )
                nc.vector.tensor_scalar_mul(
                    zd[0:C, 0:rows_l], zd[0:C, 0:rows_l], 1.0 / float(blk * blk)
                )
                nc.vector.memset(zd[C : C + 1, 0:rows_l], 1.0)
                lhs_f32 = zd
            else:
                lhs_f32 = zrest

            n_chunks = (rows_l + 127) // 128

            # hi/lo bf16 split of the augmented lhsT (3-term exact matmul)
            lhs_h = work.tile([C + 1, BS], BF16, tag="lhsh", bufs=1)
            lhs_l = work.tile([C + 1, BS], BF16, tag="lhsl", bufs=1)
            lres = work.tile([C + 1, BS], F32, tag="lres", bufs=1)
            nc.scalar.copy(lhs_h[:, 0:rows_l], lhs_f32[:, 0:rows_l])
            nc.vector.tensor_sub(
                lres[:, 0:rows_l], lhs_f32[:, 0:rows_l], lhs_h[:, 0:rows_l]
            )
            nc.scalar.copy(lhs_l[:, 0:rows_l], lres[:, 0:rows_l])
            if 0 < lvl < LEVELS - 1:
                tokT = work.tile([64, B], U32, tag="tokT", bufs=1)
                q = work.tile([64, B * C], F32, tag="q", bufs=1)
            if last:
                g4 = work.tile([128, 16 * C], F32, tag="g4", bufs=1)

            for chv in range(n_chunks):
                r0 = chv * 128
                rows = min(128, rows_l - r0)

                tok = work.tile([128, 1], U32, tag="tok")
                idxs = []
                tops = []
                for hf in range(2):
                    ps = psum.tile([128, 2048], F32, tag="ps")
                    for kk in range(4):
                        col = slice(hf * 2048 + kk * 512, hf * 2048 + (kk + 1) * 512)
                        terms = [
                            (lhs_h, w2h, True, False),
                            (lhs_h, w2l, False, False),
                            (lhs_l, w2h, False, True),
                        ]
                        for lt, wt, st, sp in terms:
                            nc.tensor.matmul(
                                ps[0:rows, kk * 512 : (kk + 1) * 512],
                                lhsT=lt[:, r0 : r0 + rows],
                                rhs=wt[:, col],
                                start=st,
                                stop=sp,
                            )
                    top8 = work.tile([128, 8], F32, tag="top8")
                    nc.vector.max(top8[0:rows, :], ps[0:rows, :])
                    idx8 = work.tile([128, 8], U32, tag="idx8")
                    nc.vector.max_index(idx8[0:rows, :], top8[0:rows, :], ps[0:rows, :])
                    idxs.append(idx8)
                    tops.append(top8)

                # combine halves, first-index tie semantics
                ge = work.tile([128, 1], mybir.dt.uint8, tag="ge")
                nc.vector.tensor_tensor(
                    ge[0:rows, :],
                    tops[0][0:rows, 0:1],
                    tops[1][0:rows, 0:1],
                    op=ALU.is_ge,
                )
                nc.vector.tensor_scalar(
                    tok[0:rows, :], idxs[1][0:rows, 0:1], 2048, None, op0=ALU.add
                )
                nc.vector.copy_predicated(
                    tok[0:rows, :], ge[0:rows, :], idxs[0][0:rows, 0:1]
                )

                if lvl == 0:
                    # 1x1 level: gather the 8 codes, transpose to [C, B],
                    # then broadcast-subtract/add over the 256 pixels.
                    g0 = work.tile([B, C], F32, tag="g")
                    nc.gpsimd.indirect_dma_start(
                        out=g0[:, :],
                        out_offset=None,
                        in_=emb_ap[:],
                        in_offset=bass.IndirectOffsetOnAxis(ap=tok[0:B, 0:1], axis=0),
                    )
                    tp = psum.tile([128, 2048], F32, tag="ps")
                    nc.tensor.transpose(tp[0:C, 0:B], g0[:, :], ident[0:B, 0:B])
                    q0t = work.tile([C, B], F32, tag="q0t")
                    nc.vector.tensor_copy(q0t[:], tp[0:C, 0:B])
                    bc = q0t[:].rearrange("c (b one) -> c b one", one=1).broadcast_to(
                        [C, B, S]
                    )
                    zr3 = zrest[0:C, :].rearrange("c (b s) -> c b s", b=B)
                    zd3 = zdec[:, :].rearrange("c (b s) -> c b s", b=B)
                    nc.vector.tensor_sub(zr3, zr3, bc)
                    nc.vector.tensor_add(zd3, zd3, bc)
                elif not last:
                    # reorder tokens [(b,pp),1] -> tokT [pp, b]
                    b_per = rows // pp
                    for bb in range(b_per):
                        b_g = chv * b_per + bb
                        nc.sync.dma_start(
                            tokT[0:pp, b_g : b_g + 1],
                            tok[bb * pp : (bb + 1) * pp, 0:1],
                        )
                else:
                    # gather this chunk's codes into the level-wide staging
                    # tile; transposes happen once after all chunks.
                    nc.gpsimd.indirect_dma_start(
                        out=g4[:, chv * C : (chv + 1) * C],
                        out_offset=None,
                        in_=emb_ap[:],
                        in_offset=bass.IndirectOffsetOnAxis(ap=tok[:, 0:1], axis=0),
                    )

            if 0 < lvl < LEVELS - 1:
                for b_g in range(B):
                    nc.gpsimd.indirect_dma_start(
                        out=q[0:pp, b_g * C : (b_g + 1) * C],
                        out_offset=None,
                        in_=emb_ap[:],
                        in_offset=bass.IndirectOffsetOnAxis(
                            ap=tokT[0:pp, b_g : b_g + 1], axis=0
                        ),
                    )
                up = psum.tile([128, 2048], F32, tag="ps")
                for b_g in range(B):
                    nc.tensor.matmul(
                        up[0:C, b_g * S : (b_g + 1) * S],
                        lhsT=q[0:pp, b_g * C : (b_g + 1) * C],
                        rhs=wup[pn][:],
                        start=True,
                        stop=True,
                    )
                nc.vector.tensor_sub(zrest[0:C, :], zrest[0:C, :], up[0:C, :])
                nc.vector.tensor_add(zdec[:, :], zdec[:, :], up[0:C, :])
            elif last:
                tp = psum.tile([128, 2048], F32, tag="ps")
                for chv in range(16):
                    nc.tensor.transpose(
                        tp[0:C, chv * 128 : (chv + 1) * 128],
                        g4[:, chv * C : (chv + 1) * C],
                        ident[:],
                    )
                nc.vector.tensor_add(zdec[:, :], zdec[:, :], tp[0:C, :])

            nc.sync.dma_start(
                out_ap[lvl : lvl + 1].rearrange("o b c s -> c (o b) s"),
                zdec[:].rearrange("c (b s) -> c b s", b=B),
            )


# ---------------------------------------------------------------------------
# Host entry point
# ---------------------------------------------------------------------------
def _build_nc():
    nc = bacc.Bacc("TRN2", target_bir_lowering=False, debug=False)
    z_t = nc.dram_tensor("z", [B, C, S], F32, kind="ExternalInput")
    emb_t = nc.dram_tensor("emb", [K_CODES, C], F32, kind="ExternalInput")
    ident_t = nc.dram_tensor("ident", [128, 128], F32, kind="ExternalInput")
    wup_ts = {
        pn: nc.dram_tensor(f"wup{pn}", [pn * pn, S], F32, kind="ExternalInput")
        for pn in MS[1:-1]
    }
    out_t = nc.dram_tensor("out", [LEVELS, B, C, S], F32, kind="ExternalOutput")

    with tile.TileContext(nc) as tc:
        build_vq_kernel(
            tc,
            out_t.ap(),
            z_t.ap(),
            emb_t.ap(),
            {pn: t.ap() for pn, t in wup_ts.items()},
            ident_t.ap(),
        )
    nc.compile()
    return nc


def kernel(z_enc: np.ndarray, emb_weight: np.ndarray) -> np.ndarray:
    z_enc = np.ascontiguousarray(z_enc, dtype=np.float32)
    emb_weight = np.ascontiguousarray(emb_weight, dtype=np.float32)
    assert z_enc.shape == (B_FULL, C, H, W)
    assert emb_weight.shape == (K_CODES, C)

    nc = _build_nc()

    ident = np.eye(128, dtype=np.float32)
    wups = {pn: _wup_matrix(pn) for pn in MS[1:-1]}

    z_flat = z_enc.reshape(B_FULL, C, S)
    in_maps = []
    for core in range(N_CORES):
        shard = np.ascontiguousarray(z_flat[core * B : (core + 1) * B])
        m = {"z": shard, "emb": emb_weight, "ident": ident}
        for pn in MS[1:-1]:
            m[f"wup{pn}"] = wups[pn]
        in_maps.append(m)

    res = run_bass_kernel_spmd(nc, in_maps, list(range(N_CORES)), trace=TRACE)
    if TRACE:
        global LAST_RESULTS
        LAST_RESULTS = res
    outs = [res.results[core]["out"] for core in range(N_CORES)]
    full = np.concatenate(outs, axis=1)  # [5, 64, 32, 256]
    return full.reshape(LEVELS, B_FULL, C, H, W)


if __name__ == "__main__":
    z = np.random.randn(B_FULL, C, H, W).astype(np.float32)
    emb = (np.random.randn(K_CODES, C) * 0.01).astype(np.float32)
    out = kernel(z, emb)
    print("kernel output:", out.shape, out.dtype)


# revision 44
# speedup vs baseline: 1.5859x; 1.0719x over previous
"""Trainium2 Bass kernel for nn_BaseQuantizer (multiscale residual VQ).

Data-parallel over batch B=64 across 8 NeuronCores (8 images per core).
Per core, for each of 5 levels (pn in 1,2,4,8,16):
  - area-downsample residual (exact block mean) via DVE strided reduces
  - distance argmin over 4096 codes: PE matmul computes
      s[r,k] = zd[r]·w[k] - |w[k]|^2/2   (argmax s == argmin L2 dist)
    via an augmented K=33 contraction ([zd;1] x [w^T; -wsq/2]); DVE
    max/max_index extract the first-index argmax per row.
  - code gather via indirect DMA from the replicated codebook in DRAM
  - bicubic upsample via PE matmuls against precomputed (A (x) A) maps
  - residual update + accumulation, ret[l] written out per level.
"""

import os
import sys

import numpy as np

if "/opt/trn_rl_repo" not in sys.path:
    sys.path.insert(0, "/opt/trn_rl_repo")

import concourse.bass as bass
import concourse.bacc as bacc
import concourse.mybir as mybir
import concourse.tile as tile
from concourse.bass_utils import run_bass_kernel_spmd

F32 = mybir.dt.float32
F32R = mybir.dt.float32r  # full-rate (1 cyc/col) exact-fp32 matmul path
U32 = mybir.dt.uint32
AX = mybir.AxisListType
ALU = mybir.AluOpType
ACTF = mybir.ActivationFunctionType

TRACE = False  # set True (e.g. from test.py) to capture an NTFF profile
ROW_TILE = os.environ.get("VQ_ROW_TILE", "1") == "1"
LAST_RESULTS = None

N_CORES = 8
B_FULL = 64
B = B_FULL // N_CORES  # 8 images per core
C = 32
H = W = 16
S = H * W  # 256
K_CODES = 4096
MS = (1, 2, 4, 8, 16)
LEVELS = len(MS)


# ---------------------------------------------------------------------------
# Host-side constants: bicubic (jax.image.resize method='cubic') weight maps.
# ---------------------------------------------------------------------------
def _keys_cubic(x: np.ndarray, a: float = -0.5) -> np.ndarray:
    x = np.abs(x)
    out = np.where(
        x <= 1.0,
        ((a + 2.0) * x - (a + 3.0)) * x * x + 1.0,
        np.where(x < 2.0, (((x - 5.0) * x + 8.0) * x - 4.0) * a, 0.0),
    )
    return out


def _resize_weight_mat(in_size: int, out_size: int) -> np.ndarray:
    """Replicates jax.image.compute_weight_mat for method='cubic' upsampling.

    Returns [in_size, out_size]; out = wm.T @ in along the resized axis.
    """
    scale = out_size / in_size
    sample_f = (np.arange(out_size, dtype=np.float64) + 0.5) / scale - 0.5
    x = np.abs(sample_f[None, :] - np.arange(in_size, dtype=np.float64)[:, None])
    weights = _keys_cubic(x)
    total = weights.sum(axis=0)
    weights = np.where(total[None, :] > 0, weights / total[None, :], 0.0)
    in_bounds = (sample_f >= -0.5) & (sample_f <= in_size - 0.5)
    weights = weights * in_bounds[None, :]
    return weights.astype(np.float32)


def _wup_matrix(pn: int) -> np.ndarray:
    """[pn*pn, 256] map: z_up[(y,x)] = sum_{(i,j)} q[(i,j)] * Wup[(i,j),(y,x)]."""
    wm = _resize_weight_mat(pn, H)  # [pn, 16]
    wup = np.einsum("iy,jx->ijyx", wm, wm).reshape(pn * pn, H * W)
    return np.ascontiguousarray(wup.astype(np.float32))


# ---------------------------------------------------------------------------
# Device kernel
# ---------------------------------------------------------------------------
def build_vq_kernel(tc, out_ap, z_ap, emb_ap, wt_ap, wup_aps, ident_ap):
    """out_ap: [5, B, 32, 256]; z_ap: [B, 32, 256]; emb_ap: [4096, 32];
    wup_aps: dict pn -> [pn*pn, 256]; ident_ap: [128, 128]."""
    nc = tc.nc
    BF16 = mybir.dt.bfloat16
    BS = B * S  # 2048

    from contextlib import ExitStack

    with ExitStack() as ctx:
        const = ctx.enter_context(tc.tile_pool(name="const", bufs=1))
        psum = ctx.enter_context(tc.tile_pool(name="psum", bufs=2, space="PSUM"))
        work = ctx.enter_context(tc.tile_pool(name="work", bufs=2))

        # Persistent state tiles
        w2h = const.tile([97, K_CODES], BF16, tag="w2h")  # bf16 hi; dup @64
        w2l = const.tile([97, K_CODES], BF16, tag="w2l")  # bf16 lo; dup @64
        zrest = const.tile([C + 1, BS], F32, tag="zrest")      # row 32 == 1.0
        zdec = const.tile([C, BS], F32, tag="zdec")
        ident = const.tile([128, 128], F32, tag="ident")
        wup = {
            pn: const.tile([pn * pn, S], F32, tag=f"wup{pn}", name=f"wup{pn}")
            for pn in MS[1:-1]
        }
        neghalf = const.tile([C, 1], F32, tag="neghalf")

        # Loads and initialization
        nc.sync.dma_start(ident[:], ident_ap[:])
        for pn in MS[1:-1]:
            nc.sync.dma_start(wup[pn][:], wup_aps[pn][:])
        nc.sync.dma_start(
            zrest[0:C, :].rearrange("c (b s) -> c b s", b=B),
            z_ap.rearrange("b c s -> c b s"),
        )
        nc.vector.memset(zrest[C : C + 1, :], 1.0)
        nc.vector.memset(zdec[:], 0.0)
        nc.vector.memset(neghalf[:], -0.5)

        # --- codebook setup: wT arrives pre-transposed from the host --------
        wtf = const.tile([C + 1, K_CODES], F32, tag="wtf")
        nc.sync.dma_start(wtf[0:C, :], wt_ap[:])

        # wsq row via ACT square + fp32 PE ones-matmul (exact)
        sq = work.tile([C, K_CODES], F32, tag="sq", bufs=1)
        nc.scalar.activation(sq[:], wtf[0:C, :], ACTF.Square)
        for h in range(2):
            ps = psum.tile([128, 2048], F32, tag="ps")
            for kk in range(4):
                nc.tensor.matmul(
                    ps[0:1, kk * 512 : (kk + 1) * 512],
                    lhsT=neghalf[:],
                    rhs=sq[:, h * 2048 + kk * 512 : h * 2048 + (kk + 1) * 512],
                    start=True,
                    stop=True,
                )
            nc.scalar.copy(wtf[C : C + 1, h * 2048 : (h + 1) * 2048], ps[0:1, :])

        # hi/lo bf16 split of the augmented codebook (+ base-64 duplicates)
        nc.scalar.copy(w2h[0 : C + 1, :], wtf[:, :])
        wres = work.tile([C + 1, K_CODES], F32, tag="wres", bufs=1)
        nc.vector.tensor_sub(wres[:, :], wtf[:, :], w2h[0 : C + 1, :])
        nc.scalar.copy(w2l[0 : C + 1, :], wres[:, :])
        nc.sync.dma_start(w2h[64 : 64 + C + 1, :], w2h[0 : C + 1, :])
        nc.sync.dma_start(w2l[64 : 64 + C + 1, :], w2l[0 : C + 1, :])

        # --- level loop ----------------------------------------------------
        for lvl, pn in enumerate(MS):
            pp = pn * pn
            rows_l = B * pp
            blk = H // pn
            last = lvl == LEVELS - 1

            if not last:
                # exact area pooling: two strided sum-reduces + scale
                zd = work.tile([C + 1, 512], F32, tag="zd", bufs=1)
                t1 = work.tile([C, 128 * pn], F32, tag="t1", bufs=1)
                nc.vector.reduce_sum(
                    t1[:, : 128 * pn],
                    zrest[0:C, :].rearrange(
                        "c (bh pw bw) -> c bh pw bw", pw=pn, bw=blk
                    ),
                    axis=AX.X,
                )
                nc.vector.reduce_sum(
                    zd[0:C, 0:rows_l],
                    t1[:, : 128 * pn].rearrange(
                        "c (b ph bh2 pw) -> c b ph pw bh2", ph=pn, bh2=blk, pw=pn
                    ),
                    axis=AX.X,
                )
                nc.vector.tensor_scalar_mul(
                    zd[0:C, 0:rows_l], zd[0:C, 0:rows_l], 1.0 / float(blk * blk)
                )
                nc.vector.memset(zd[C : C + 1, 0:rows_l], 1.0)
                lhs_f32 = zd
            else:
                lhs_f32 = zrest

            n_chunks = (rows_l + 127) // 128

            exact_lvl = lvl in (0, 2)
            # hi/lo bf16 split of the augmented lhsT (3-term exact matmul)
            lhs_h = work.tile([97, BS], BF16, tag="lhsh", bufs=1)
            lhs_l = work.tile([97, BS], BF16, tag="lhsl", bufs=1)
            lres = work.tile([C + 1, BS], F32, tag="lres", bufs=1)
            if not exact_lvl:
                nc.scalar.copy(lhs_h[0 : C + 1, 0:rows_l], lhs_f32[:, 0:rows_l])
                nc.vector.tensor_sub(
                    lres[:, 0:rows_l], lhs_f32[:, 0:rows_l], lhs_h[0 : C + 1, 0:rows_l]
                )
                nc.scalar.copy(lhs_l[0 : C + 1, 0:rows_l], lres[:, 0:rows_l])
                nc.sync.dma_start(
                    lhs_h[64 : 64 + C + 1, 0:rows_l], lhs_h[0 : C + 1, 0:rows_l]
                )
                nc.sync.dma_start(
                    lhs_l[64 : 64 + C + 1, 0:rows_l], lhs_l[0 : C + 1, 0:rows_l]
                )
            if 0 < lvl < LEVELS - 1:
                tokT = work.tile([64, B], U32, tag="tokT", bufs=1)
                q = work.tile([64, B * C], F32, tag="q", bufs=1)
            if last:
                g4 = work.tile([128, 16 * C], F32, tag="g4", bufs=1)

            for chv in range(n_chunks):
                r0 = chv * 128
                rows = min(128, rows_l - r0)

                tok = work.tile([128, 1], U32, tag="tok")
                idxs = []
                tops = []
                for hf in range(2):
                    pb = 64 * hf if (ROW_TILE and not exact_lvl) else 0
                    ps = psum.tile([128, 2048], F32, tag="ps")
                    for kk in range(4):
                        col = slice(hf * 2048 + kk * 512, hf * 2048 + (kk + 1) * 512)
                        if exact_lvl:
                            nc.tensor.matmul(
                                ps[0:rows, kk * 512 : (kk + 1) * 512],
                                lhsT=lhs_f32[:, r0 : r0 + rows],
                                rhs=wtf[:, col],
                                start=True,
                                stop=True,
                            )
                            continue
                        terms = [
                            (lhs_h, w2h, True, False),
                            (lhs_h, w2l, False, False),
                            (lhs_l, w2h, False, True),
                        ]
                        for lt, wt, st, sp in terms:
                            nc.tensor.matmul(
                                ps[0:rows, kk * 512 : (kk + 1) * 512],
                                lhsT=lt[pb : pb + C + 1, r0 : r0 + rows],
                                rhs=wt[pb : pb + C + 1, col],
                                start=st,
                                stop=sp,
                                tile_position=(pb, 0) if (ROW_TILE and not exact_lvl) else None,
                            )
                    top8 = work.tile([128, 8], F32, tag="top8")
                    nc.vector.max(top8[0:rows, :], ps[0:rows, :])
                    idx8 = work.tile([128, 8], U32, tag="idx8")
                    nc.vector.max_index(idx8[0:rows, :], top8[0:rows, :], ps[0:rows, :])
                    idxs.append(idx8)
                    tops.append(top8)

                # combine halves, first-index tie semantics
                ge = work.tile([128, 1], mybir.dt.uint8, tag="ge")
                nc.vector.tensor_tensor(
                    ge[0:rows, :],
                    tops[0][0:rows, 0:1],
                    tops[1][0:rows, 0:1],
                    op=ALU.is_ge,
                )
                nc.vector.tensor_scalar(
                    tok[0:rows, :], idxs[1][0:rows, 0:1], 2048, None, op0=ALU.add
                )
                nc.vector.copy_predicated(
                    tok[0:rows, :], ge[0:rows, :], idxs[0][0:rows, 0:1]
                )

                if lvl == 0:
                    # 1x1 level: gather the 8 codes, transpose to [C, B],
                    # then broadcast-subtract/add over the 256 pixels.
                    g0 = work.tile([B, C], F32, tag="g")
                    nc.gpsimd.indirect_dma_start(
                        out=g0[:, :],
                        out_offset=None,
                        in_=emb_ap[:],
                        in_offset=bass.IndirectOffsetOnAxis(ap=tok[0:B, 0:1], axis=0),
                    )
                    tp = psum.tile([128, 2048], F32, tag="ps")
                    nc.tensor.transpose(tp[0:C, 0:B], g0[:, :], ident[0:B, 0:B])
                    q0t = work.tile([C, B], F32, tag="q0t")
                    nc.vector.tensor_copy(q0t[:], tp[0:C, 0:B])
                    bc = q0t[:].rearrange("c (b one) -> c b one", one=1).broadcast_to(
                        [C, B, S]
                    )
                    zr3 = zrest[0:C, :].rearrange("c (b s) -> c b s", b=B)
                    zd3 = zdec[:, :].rearrange("c (b s) -> c b s", b=B)
                    nc.vector.tensor_sub(zr3, zr3, bc)
                    nc.vector.tensor_add(zd3, zd3, bc)
                elif not last:
                    # reorder tokens [(b,pp),1] -> tokT [pp, b]
                    b_per = rows // pp
                    for bb in range(b_per):
                        b_g = chv * b_per + bb
                        nc.sync.dma_start(
                            tokT[0:pp, b_g : b_g + 1],
                            tok[bb * pp : (bb + 1) * pp, 0:1],
                        )
                else:
                    # gather this chunk's codes into the level-wide staging
                    # tile; transposes happen once after all chunks.
                    nc.gpsimd.indirect_dma_start(
                        out=g4[:, chv * C : (chv + 1) * C],
                        out_offset=None,
                        in_=emb_ap[:],
                        in_offset=bass.IndirectOffsetOnAxis(ap=tok[:, 0:1], axis=0),
                    )

            if 0 < lvl < LEVELS - 1:
                for b_g in range(B):
                    nc.gpsimd.indirect_dma_start(
                        out=q[0:pp, b_g * C : (b_g + 1) * C],
                        out_offset=None,
                        in_=emb_ap[:],
                        in_offset=bass.IndirectOffsetOnAxis(
                            ap=tokT[0:pp, b_g : b_g + 1], axis=0
                        ),
                    )
                up = psum.tile([128, 2048], F32, tag="ps")
                for b_g in range(B):
                    nc.tensor.matmul(
                        up[0:C, b_g * S : (b_g + 1) * S],
                        lhsT=q[0:pp, b_g * C : (b_g + 1) * C],
                        rhs=wup[pn][:],
                        start=True,
                        stop=True,
                    )
                nc.vector.tensor_sub(zrest[0:C, :], zrest[0:C, :], up[0:C, :])
                nc.vector.tensor_add(zdec[:, :], zdec[:, :], up[0:C, :])
            elif last:
                tp = psum.tile([128, 2048], F32, tag="ps")
                for chv in range(16):
                    nc.tensor.transpose(
                        tp[0:C, chv * 128 : (chv + 1) * 128],
                        g4[:, chv * C : (chv + 1) * C],
                        ident[:],
                    )
                nc.vector.tensor_add(zdec[:, :], zdec[:, :], tp[0:C, :])

            nc.sync.dma_start(
                out_ap[lvl : lvl + 1].rearrange("o b c s -> c (o b) s"),
                zdec[:].rearrange("c (b s) -> c b s", b=B),
            )


# ---------------------------------------------------------------------------
# Host entry point
# ---------------------------------------------------------------------------
def _build_nc():
    nc = bacc.Bacc("TRN2", target_bir_lowering=False, debug=False)
    z_t = nc.dram_tensor("z", [B, C, S], F32, kind="ExternalInput")
    emb_t = nc.dram_tensor("emb", [K_CODES, C], F32, kind="ExternalInput")
    wt_t = nc.dram_tensor("wt", [C, K_CODES], F32, kind="ExternalInput")
    ident_t = nc.dram_tensor("ident", [128, 128], F32, kind="ExternalInput")
    wup_ts = {
        pn: nc.dram_tensor(f"wup{pn}", [pn * pn, S], F32, kind="ExternalInput")
        for pn in MS[1:-1]
    }
    out_t = nc.dram_tensor("out", [LEVELS, B, C, S], F32, kind="ExternalOutput")

    with tile.TileContext(nc) as tc:
        build_vq_kernel(
            tc,
            out_t.ap(),
            z_t.ap(),
            emb_t.ap(),
            wt_t.ap(),
            {pn: t.ap() for pn, t in wup_ts.items()},
            ident_t.ap(),
        )
    nc.compile()
    return nc


def kernel(z_enc: np.ndarray, emb_weight: np.ndarray) -> np.ndarray:
    z_enc = np.ascontiguousarray(z_enc, dtype=np.float32)
    emb_weight = np.ascontiguousarray(emb_weight, dtype=np.float32)
    assert z_enc.shape == (B_FULL, C, H, W)
    assert emb_weight.shape == (K_CODES, C)

    nc = _build_nc()

    ident = np.eye(128, dtype=np.float32)
    wt = np.ascontiguousarray(emb_weight.T)
    wups = {pn: _wup_matrix(pn) for pn in MS[1:-1]}

    z_flat = z_enc.reshape(B_FULL, C, S)
    in_maps = []
    for core in range(N_CORES):
        shard = np.ascontiguousarray(z_flat[core * B : (core + 1) * B])
        m = {"z": shard, "emb": emb_weight, "wt": wt, "ident": ident}
        for pn in MS[1:-1]:
            m[f"wup{pn}"] = wups[pn]
        in_maps.append(m)

    res = run_bass_kernel_spmd(nc, in_maps, list(range(N_CORES)), trace=TRACE)
    if TRACE:
        global LAST_RESULTS
        LAST_RESULTS = res
    outs = [res.results[core]["out"] for core in range(N_CORES)]
    full = np.concatenate(outs, axis=1)  # [5, 64, 32, 256]
    return full.reshape(LEVELS, B_FULL, C, H, W)


if __name__ == "__main__":
    z = np.random.randn(B_FULL, C, H, W).astype(np.float32)
    emb = (np.random.randn(K_CODES, C) * 0.01).astype(np.float32)
    out = kernel(z, emb)
    print("kernel output:", out.shape, out.dtype)
